# revision 2
# baseline (speedup 1.0000x reference)
"""Trainium2 Bass kernel for nn_CCL_Module (3x3 cost-volume softmax flow).

Reference computation (per batch):
  c1 = l2norm_C(feature1); wp = l2norm_C(feature2) zero-padded spatially.
  match_vol[d=(dh,dw)] = sum_C c1 * shift(wp, dh, dw)      (9 shifts, 3x3)
  p = softmax(10 * match_vol, over d)
  flow_w = sum_d p * dw ; flow_h = sum_d p * dh
  out = concat([flow_w, flow_h])  -> [B, 2, H, W]

Strategy (pure data parallel, one batch per NeuronCore, 8 cores):
  - SBUF layout: H=128 on partitions, free dims = (C=64, W).
  - dh shifts  -> three h-shifted copies of feature2 loaded by DMA.
  - dw shifts  -> free-dim AP offsets into w-padded tiles.
  - Raw (unnormalized) dots A_d = sum_C f1 * shift(f2) via DVE
    tensor_mul + strided tensor_reduce (reduce innermost = C).
  - L2 normalization folded into score scaling:
      score_d = 10 * A_d * rsqrt(|f1|^2) * rsqrt(|f2|^2 shifted)
  - Scores are bounded by |10| so softmax needs no max subtraction:
      flow = (sum_d w_d * exp(s_d)) / (sum_d exp(s_d))
"""

import numpy as np

B, C, H, W = 8, 64, 128, 128
N_CORES = 8
SOFTMAX_SCALE = 10.0

_CACHE = {}


def _build_program(repeat: int = 1, variant: str = "full"):
    import concourse.bass as bass
    import concourse.bacc as bacc
    import concourse.mybir as mybir
    from concourse.tile import TileContext
    from concourse.bass_utils import axon_active

    f32 = mybir.dt.float32
    nc = bacc.Bacc(
        "TRN2",
        target_bir_lowering=False,
        debug=not axon_active(),
        num_devices=N_CORES,
    )

    f1d = nc.declare_dram_parameter("feature1", [C, H, W], f32, isOutput=False)
    f2d = nc.declare_dram_parameter("feature2", [C, H, W], f32, isOutput=False)
    outd = nc.declare_dram_parameter("flow", [2, H, W], f32, isOutput=True)

    # DRAM views with h on the outer (partition) axis.
    f1v = f1d.rearrange("c h w -> h c w")
    f2v = f2d.rearrange("c h w -> h c w")
    outv = outd.rearrange("c h w -> h c w")

    # all-zero row used to zero-fill the dh edge partitions at load time
    zrow = nc.inline_tensor(np.zeros((1, C, W + 2), dtype=np.float32), name="zrow")

    with TileContext(nc) as tc:
        with tc.tile_pool(name="main", bufs=1) as pool:
          for _rep in range(repeat):
            # ---- input tiles ----
            xf1 = pool.tile([H, C, W], f32)          # f1, no padding
            # f2 with w padding (cols 0 and W+1), one tile per dh in {-1,0,1}.
            xf2_m = pool.tile([H, C, W + 2], f32)
            xf2_0 = pool.tile([H, C, W + 2], f32)
            xf2_p = pool.tile([H, C, W + 2], f32)

            nc.sync.dma_start(out=xf1[:, :, :], in_=f1v)
            # dh=0
            nc.sync.dma_start(out=xf2_0[:, :, 1 : W + 1], in_=f2v)
            # dh=-1: partition p holds f2 row p-1; row 0 is out of bounds -> 0
            nc.sync.dma_start(out=xf2_m[1:H, :, 1 : W + 1], in_=f2v[0 : H - 1])
            nc.sync.dma_start(out=xf2_m[0:1, :, :], in_=zrow[:])
            # dh=+1: partition p holds f2 row p+1; row H-1 out of bounds -> 0
            nc.sync.dma_start(out=xf2_p[0 : H - 1, :, 1 : W + 1], in_=f2v[1:H])
            nc.sync.dma_start(out=xf2_p[H - 1 : H, :, :], in_=zrow[:])

            # zero the w-pad columns so dw edge dots are exactly 0
            # (edge partitions already fully zeroed above; partition-0-based
            # memsets are legal for compute engines)
            for t in (xf2_m, xf2_0, xf2_p):
                nc.vector.memset(t[:, :, 0:1], 0.0)
                nc.vector.memset(t[:, :, W + 1 : W + 2], 0.0)

            xf2 = [xf2_m, xf2_0, xf2_p]

            # ---- raw correlation dots ----
            prod = pool.tile([H, C, W], f32)
            scoresA = pool.tile([H, 9, W], f32)     # A_d, d = dh*3+dw

            nmuls = 0 if variant == "loads" else 9
            for d in range(nmuls):
                dh, dw = d // 3 - 1, d % 3 - 1
                src = xf2[dh + 1][:, :, 1 + dw : 1 + dw + W]
                nc.vector.tensor_mul(prod[:, :, :], xf1[:, :, :], src)
                if variant == "muls":
                    continue
                # reduce over C (innermost after permute)
                nc.vector.tensor_reduce(
                    scoresA[:, d, :],
                    prod.rearrange("h c w -> h w c"),
                    axis=mybir.AxisListType.X,
                    op=mybir.AluOpType.add,
                )
            if variant in ("loads", "muls"):
                # consume every loaded tile so DCE can't drop the DMAs
                flows0 = pool.tile([H, 2, W], f32)
                nc.vector.tensor_add(flows0[:, 0, :], xf1[:, 0, :], xf2_m[:, 0, 0:W])
                nc.vector.tensor_add(flows0[:, 0, :], flows0[:, 0, :], xf2_0[:, 0, 0:W])
                nc.vector.tensor_add(flows0[:, 1, :], xf2_p[:, 0, 0:W], prod[:, 0, :])
                nc.sync.dma_start(out=outv, in_=flows0[:, :, :])
                continue

            # ---- norms ----
            r1sq = pool.tile([H, W], f32)
            r2m = pool.tile([H, W + 2], f32)  # |f2|^2 map, w-padded
            nc.vector.tensor_mul(prod[:, :, :], xf1[:, :, :], xf1[:, :, :])
            nc.vector.tensor_reduce(
                r1sq[:, :],
                prod.rearrange("h c w -> h w c"),
                axis=mybir.AxisListType.X,
                op=mybir.AluOpType.add,
            )
            f20 = xf2_0[:, :, 1 : W + 1]
            nc.vector.tensor_mul(prod[:, :, :], f20, f20)
            nc.vector.memset(r2m[:, 0:1], 1.0)
            nc.vector.memset(r2m[:, W + 1 : W + 2], 1.0)
            nc.vector.tensor_reduce(
                r2m[:, 1 : W + 1],
                prod.rearrange("h c w -> h w c"),
                axis=mybir.AxisListType.X,
                op=mybir.AluOpType.add,
            )

            # recip1 = 1/sqrt(r1sq), recip2 = 1/sqrt(r2m)
            recip1 = pool.tile([H, W], f32)
            recip2 = pool.tile([H, W + 2], f32)
            nc.scalar.sqrt(recip1[:, :], r1sq[:, :])
            nc.vector.reciprocal(recip1[:, :], recip1[:, :])
            nc.scalar.sqrt(recip2[:, :], r2m[:, :])
            nc.vector.reciprocal(recip2[:, :], recip2[:, :])

            # dh-shifted copies of recip2. Compute engines cannot address
            # partition-shifted APs, so shift across partitions via
            # SBUF->SBUF DMA. Edge rows clamp (their A is exactly 0).
            rec2_m = pool.tile([H, W + 2], f32)
            rec2_p = pool.tile([H, W + 2], f32)
            nc.sync.dma_start(out=rec2_m[1:H, :], in_=recip2[0 : H - 1, :])
            nc.sync.dma_start(out=rec2_m[0:1, :], in_=recip2[0:1, :])
            nc.sync.dma_start(out=rec2_p[0 : H - 1, :], in_=recip2[1:H, :])
            nc.sync.dma_start(out=rec2_p[H - 1 : H, :], in_=recip2[H - 1 : H, :])
            rec2 = [rec2_m, recip2, rec2_p]

            # ---- scores -> exp ----
            rmul = pool.tile([H, 9, W], f32)
            for d in range(9):
                dh, dw = d // 3 - 1, d % 3 - 1
                nc.vector.tensor_mul(
                    rmul[:, d, :], recip1[:, :], rec2[dh + 1][:, 1 + dw : 1 + dw + W]
                )
            expo = pool.tile([H, 9, W], f32)
            nc.vector.tensor_mul(rmul[:, :, :], rmul[:, :, :], scoresA[:, :, :])
            nc.scalar.activation(
                expo[:, :, :],
                rmul[:, :, :],
                mybir.ActivationFunctionType.Exp,
                scale=SOFTMAX_SCALE,
            )

            # ---- softmax-weighted displacement sums ----
            esum = pool.tile([H, W], f32)
            fwp = pool.tile([H, W], f32)
            fwm = pool.tile([H, W], f32)
            fhp = pool.tile([H, W], f32)
            fhm = pool.tile([H, W], f32)
            ex4 = expo.rearrange("h (a b) w -> h a b w", a=3)
            red = dict(axis=mybir.AxisListType.X, op=mybir.AluOpType.add)
            nc.vector.tensor_reduce(
                esum[:, :], expo.rearrange("h d w -> h w d"), **red
            )
            nc.vector.tensor_reduce(
                fwp[:, :], ex4[:, :, 2, :].rearrange("h a w -> h w a"), **red
            )
            nc.vector.tensor_reduce(
                fwm[:, :], ex4[:, :, 0, :].rearrange("h a w -> h w a"), **red
            )
            nc.vector.tensor_reduce(
                fhp[:, :], ex4[:, 2, :, :].rearrange("h b w -> h w b"), **red
            )
            nc.vector.tensor_reduce(
                fhm[:, :], ex4[:, 0, :, :].rearrange("h b w -> h w b"), **red
            )

            flows = pool.tile([H, 2, W], f32)
            nc.vector.reciprocal(esum[:, :], esum[:, :])
            nc.vector.tensor_sub(fwp[:, :], fwp[:, :], fwm[:, :])
            nc.vector.tensor_sub(fhp[:, :], fhp[:, :], fhm[:, :])
            nc.vector.tensor_mul(flows[:, 0, :], fwp[:, :], esum[:, :])
            nc.vector.tensor_mul(flows[:, 1, :], fhp[:, :], esum[:, :])

            nc.sync.dma_start(out=outv, in_=flows[:, :, :])

    nc.compile()
    return nc


LAST_RESULT = None


def kernel(feature1: np.ndarray, feature2: np.ndarray) -> np.ndarray:
    global LAST_RESULT
    from concourse import bass_utils

    if "nc" not in _CACHE:
        _CACHE["nc"] = _build_program()
    nc = _CACHE["nc"]

    f1 = np.ascontiguousarray(np.asarray(feature1, dtype=np.float32))
    f2 = np.ascontiguousarray(np.asarray(feature2, dtype=np.float32))
    in_maps = [
        {"feature1": f1[b], "feature2": f2[b]} for b in range(N_CORES)
    ]
    res = bass_utils.run_bass_kernel_spmd(nc, in_maps, list(range(N_CORES)))
    LAST_RESULT = res
    out = np.stack([res.results[b]["flow"] for b in range(N_CORES)], axis=0)
    return out.astype(np.float32)



# revision 6
# speedup vs baseline: 6.3163x; 6.3163x over previous
"""Trainium2 Bass kernel for nn_CCL_Module (3x3 cost-volume softmax flow).

Reference computation (per batch):
  c1 = l2norm_C(feature1); wp = l2norm_C(feature2) zero-padded spatially.
  match_vol[d=(dh,dw)] = sum_C c1 * shift(wp, dh, dw)      (9 shifts, 3x3)
  p = softmax(10 * match_vol, over d)
  flow_w = sum_d p * dw ; flow_h = sum_d p * dh
  out = concat([flow_w, flow_h])  -> [B, 2, H, W]

Strategy (pure data parallel, one batch per NeuronCore, 8 cores):
  - SBUF layout: partitions = (hh, c) with hh the H-half (2) and c the
    channel (64); free dim = (h', w) flat (64*128 = 8192).  Loads from
    DRAM are 128 contiguous 32KB descriptors with SWDGE f32->fp16 cast.
  - All 9 (dh, dw) shifts are free-dim offsets delta = 128*dh + dw into a
    zero-padded copy of feature2.  A second copy shifted by one element
    keeps odd-delta operands 4-byte aligned so DVE fp16 muls run at 2x.
  - Channel reduction via TensorE: matmul with a sliding one-hot
    column-pair mask [128, 128] (ones over partitions 0-63 in col 2k,
    over 64-127 in col 2k+1) reduces chunk k of a product over c and
    drops row sums into PSUM rows (2k, 2k+1); 64 accumulating matmuls
    per map fill a [128, 128] fp32 score tile laid out as
    [p = 2h'+hh, w].  11 maps: 9 correlations + |f1|^2 + |f2|^2.
  - L2 normalization folded into score scaling:
      score_d = 10 * A_d * rsqrt(|f1|^2) * rsqrt(|f2|^2 shifted)
    Scores bounded by 10 so softmax needs no max subtraction.
"""

import numpy as np

B, C, H, W = 8, 64, 128, 128
N_CORES = 8
SOFTMAX_SCALE = 10.0

HH = 2
HP = H // HH            # 64 h-rows per half
FREE = HP * W           # 8192 free elems per partition
M0 = 160                # main-data column offset in padded f2 tiles
F2W = M0 + FREE + 160   # 8512

_CACHE = {}


def _build_program():
    import concourse.bass as bass
    import concourse.bacc as bacc
    import concourse.mybir as mybir
    from concourse.tile import TileContext
    from concourse.bass_utils import axon_active

    f32 = mybir.dt.float32
    f16 = mybir.dt.float16
    AF = mybir.ActivationFunctionType
    red = dict(axis=mybir.AxisListType.X, op=mybir.AluOpType.add)

    nc = bacc.Bacc(
        "TRN2",
        target_bir_lowering=False,
        debug=not axon_active(),
        num_devices=N_CORES,
    )

    f1d = nc.declare_dram_parameter("feature1", [C, H, W], f32, isOutput=False)
    f2d = nc.declare_dram_parameter("feature2", [C, H, W], f32, isOutput=False)
    outd = nc.declare_dram_parameter("flow", [2, H, W], f32, isOutput=True)

    # [64, 8192] per-half views: partition c, free = h'*W + w
    f1h = [
        f1d[:, hh * HP : (hh + 1) * HP, :].rearrange("c hp w -> c (hp w)")
        for hh in range(HH)
    ]
    f2h = [
        f2d[:, hh * HP : (hh + 1) * HP, :].rearrange("c hp w -> c (hp w)")
        for hh in range(HH)
    ]
    # output view: partition = h (score layout), free = (o, w)
    outv = outd.rearrange("o h w -> h o w")

    with TileContext(nc) as tc:
        with tc.tile_pool(name="main", bufs=1) as pool, \
             tc.tile_pool(name="prod", bufs=3) as prodp, \
             tc.tile_pool(name="psum", bufs=1, space="PSUM") as psp:

            # ---- input tiles (fp16, cast during SWDGE DMA) ----
            xf1 = pool.tile([128, FREE], f16)
            xf2 = pool.tile([128, F2W], f16)    # even-parity padded f2
            xf2o = pool.tile([128, F2W], f16)   # same, shifted 1 elem left

            nc.gpsimd.dma_start(out=xf1[0:64, :], in_=f1h[0])
            nc.gpsimd.dma_start(out=xf1[64:128, :], in_=f1h[1])

            # zero the pad regions (h'=-1 / h'=64 halos + spare)
            nc.vector.memset(xf2[:, 0:M0], 0.0)
            nc.vector.memset(xf2[:, M0 + FREE : F2W], 0.0)
            nc.vector.memset(xf2o[:, 0 : M0 - 1], 0.0)
            nc.vector.memset(xf2o[:, M0 - 1 + FREE : F2W], 0.0)

            # main f2 data
            nc.gpsimd.dma_start(out=xf2[0:64, M0 : M0 + FREE], in_=f2h[0])
            nc.gpsimd.dma_start(out=xf2[64:128, M0 : M0 + FREE], in_=f2h[1])
            nc.gpsimd.dma_start(
                out=xf2o[0:64, M0 - 1 : M0 - 1 + FREE], in_=f2h[0]
            )
            nc.gpsimd.dma_start(
                out=xf2o[64:128, M0 - 1 : M0 - 1 + FREE], in_=f2h[1]
            )
            # halo rows: hh=0 needs h=64 above its top edge is h=-1 (zero),
            # below its last row h'=63 is h=64; hh=1 has h=63 above, zero below.
            nc.gpsimd.dma_start(
                out=xf2[64:128, M0 - 128 : M0], in_=f2d[:, HP - 1, :]
            )
            nc.gpsimd.dma_start(
                out=xf2[0:64, M0 + FREE : M0 + FREE + 128], in_=f2d[:, HP, :]
            )
            nc.gpsimd.dma_start(
                out=xf2o[64:128, M0 - 129 : M0 - 1], in_=f2d[:, HP - 1, :]
            )
            nc.gpsimd.dma_start(
                out=xf2o[0:64, M0 - 1 + FREE : M0 - 1 + FREE + 128],
                in_=f2d[:, HP, :],
            )

            # ---- sliding one-hot mask for the channel-reduce matmuls ----
            # S_k = smask[:, 63-k : 191-k]:
            #   S_k[p, k]    = 1 for p in [0,64)   -> PSUM row k    = h = k
            #   S_k[p, 64+k] = 1 for p in [64,128) -> row 64+k      = h = 64+k
            smask = pool.tile([128, 191], f16)
            nc.vector.memset(smask[:, :], 0.0)
            nc.vector.memset(smask[0:64, 63:64], 1.0)
            nc.vector.memset(smask[64:128, 127:128], 1.0)

            # ---- PSUM score banks: 11 maps x [128, 128] fp32 ----
            ps0 = psp.tile([128, 512], f32, tag="ps0")
            ps1 = psp.tile([128, 512], f32, tag="ps1")
            ps2 = psp.tile([128, 512], f32, tag="ps2")

            def score_slice(m):
                t = (ps0, ps1, ps2)[m // 4]
                j = (m % 4) * 128
                return t[:, j : j + 128]

            # map id: d = 3*(dh+1)+(dw+1) for the 9 shifts, 9 = |f1|^2,
            # 10 = |f2|^2.  Loop order puts early-ready maps first.
            def shift_view(dh, dw):
                delta = 128 * dh + dw
                if dw == 0:
                    return xf2[:, M0 + delta : M0 + delta + FREE]
                return xf2o[:, M0 + delta - 1 : M0 + delta - 1 + FREE]

            order = [9, 10, 4, 1, 7, 3, 5, 0, 2, 6, 8]
            for m in order:
                pr = prodp.tile([128, FREE], f16, tag="prod")
                if m == 9:
                    nc.scalar.activation(pr[:, :], xf1[:, :], AF.Square)
                elif m == 10:
                    nc.scalar.activation(
                        pr[:, :], xf2[:, M0 : M0 + FREE], AF.Square
                    )
                else:
                    dh, dw = m // 3 - 1, m % 3 - 1
                    nc.vector.tensor_mul(pr[:, :], xf1[:, :], shift_view(dh, dw))
                out = score_slice(m)
                for k in range(HP):
                    nc.tensor.matmul(
                        out,
                        smask[:, 63 - k : 191 - k],
                        pr[:, 128 * k : 128 * (k + 1)],
                        start=(k == 0),
                        stop=(k == HP - 1),
                    )

            # ---- gather scores to SBUF: cols d*128 (d=0..8), n1@1152, n2@1280
            s_all = pool.tile([128, 1408], f32)
            nc.vector.tensor_copy(s_all[:, 0:512], ps0[:, :])
            nc.vector.tensor_copy(s_all[:, 512:1024], ps1[:, :])
            nc.scalar.copy(s_all[:, 1024:1408], ps2[:, 0:384])
            sc = s_all[:, 0:1152]

            # ---- rsqrt of norms ----
            r1 = pool.tile([128, 128], f32)
            r2p = pool.tile([128, 130], f32)
            nc.scalar.sqrt(r1[:, :], s_all[:, 1152:1280])
            nc.vector.reciprocal(r1[:, :], r1[:, :])
            nc.vector.memset(r2p[:, 0:1], 1.0)
            nc.vector.memset(r2p[:, 129:130], 1.0)
            nc.scalar.sqrt(r2p[:, 1:129], s_all[:, 1280:1408])
            nc.vector.reciprocal(r2p[:, 1:129], r2p[:, 1:129])

            # dh-shifted copies of r2 (partition = h, shift by one).
            # Edge rows clamp (their A is exactly 0).
            r2hp = pool.tile([128, 130], f32)
            r2hm = pool.tile([128, 130], f32)
            nc.sync.dma_start(out=r2hp[0:127, :], in_=r2p[1:128, :])
            nc.sync.dma_start(out=r2hp[127:128, :], in_=r2p[127:128, :])
            nc.sync.dma_start(out=r2hm[1:128, :], in_=r2p[0:127, :])
            nc.sync.dma_start(out=r2hm[0:1, :], in_=r2p[0:1, :])

            # scores -> normalized scores (in place):
            # shat_d[h, w] = A_d[h, w] * r1[h, w] * r2[h+dh, w+dw]
            for d in range(9):
                dh, dw = d // 3 - 1, d % 3 - 1
                r2x = (r2hm, r2p, r2hp)[dh + 1]
                sd = sc[:, d * 128 : (d + 1) * 128]
                nc.vector.tensor_mul(sd, sd, r2x[:, 1 + dw : 129 + dw])
                nc.vector.tensor_mul(sd, sd, r1[:, :])
            # w-edge shifts wrap across rows: those scores are exactly 0
            # in the reference (zero padding), so overwrite them.
            for d in (0, 3, 6):
                nc.vector.memset(sc[:, d * 128 : d * 128 + 1], 0.0)
            for d in (2, 5, 8):
                nc.vector.memset(sc[:, d * 128 + 127 : d * 128 + 128], 0.0)

            # ---- softmax-weighted displacement sums ----
            expo = pool.tile([128, 1152], f32)
            nc.scalar.activation(
                expo[:, :], sc, AF.Exp, scale=SOFTMAX_SCALE
            )

            esum = pool.tile([128, 128], f32)
            fwp = pool.tile([128, 128], f32)
            fwm = pool.tile([128, 128], f32)
            fhp = pool.tile([128, 128], f32)
            fhm = pool.tile([128, 128], f32)
            ex4 = expo.rearrange("p (a b w) -> p a b w", a=3, b=3)
            nc.vector.tensor_reduce(
                esum[:, :], expo.rearrange("p (d w) -> p w d", d=9), **red
            )
            nc.vector.tensor_reduce(
                fwp[:, :], ex4[:, :, 2, :].rearrange("p a w -> p w a"), **red
            )
            nc.vector.tensor_reduce(
                fwm[:, :], ex4[:, :, 0, :].rearrange("p a w -> p w a"), **red
            )
            nc.vector.tensor_reduce(
                fhp[:, :], ex4[:, 2, :, :].rearrange("p b w -> p w b"), **red
            )
            nc.vector.tensor_reduce(
                fhm[:, :], ex4[:, 0, :, :].rearrange("p b w -> p w b"), **red
            )

            flows = pool.tile([128, 2, W], f32)
            nc.vector.reciprocal(esum[:, :], esum[:, :])
            nc.vector.tensor_sub(fwp[:, :], fwp[:, :], fwm[:, :])
            nc.vector.tensor_sub(fhp[:, :], fhp[:, :], fhm[:, :])
            nc.vector.tensor_mul(flows[:, 0, :], fwp[:, :], esum[:, :])
            nc.vector.tensor_mul(flows[:, 1, :], fhp[:, :], esum[:, :])

            nc.sync.dma_start(out=outv, in_=flows[:, :, :])

    nc.compile()
    return nc


LAST_RESULT = None


def kernel(feature1: np.ndarray, feature2: np.ndarray) -> np.ndarray:
    global LAST_RESULT
    from concourse import bass_utils

    if "nc" not in _CACHE:
        _CACHE["nc"] = _build_program()
    nc = _CACHE["nc"]

    f1 = np.ascontiguousarray(np.asarray(feature1, dtype=np.float32))
    f2 = np.ascontiguousarray(np.asarray(feature2, dtype=np.float32))
    in_maps = [
        {"feature1": f1[b], "feature2": f2[b]} for b in range(N_CORES)
    ]
    res = bass_utils.run_bass_kernel_spmd(nc, in_maps, list(range(N_CORES)))
    LAST_RESULT = res
    out = np.stack([res.results[b]["flow"] for b in range(N_CORES)], axis=0)
    return out.astype(np.float32)


# revision 9
# speedup vs baseline: 7.3433x; 1.1626x over previous
"""Trainium2 Bass kernel for nn_CCL_Module (3x3 cost-volume softmax flow).

Reference computation (per batch):
  c1 = l2norm_C(feature1); wp = l2norm_C(feature2) zero-padded spatially.
  match_vol[d=(dh,dw)] = sum_C c1 * shift(wp, dh, dw)      (9 shifts, 3x3)
  p = softmax(10 * match_vol, over d)
  flow_w = sum_d p * dw ; flow_h = sum_d p * dh
  out = concat([flow_w, flow_h])  -> [B, 2, H, W]

Strategy (pure data parallel, one batch per NeuronCore, 8 cores):
  - SBUF layout: partitions = (hh, c) with hh the H-half (2) and c the
    channel (64); free dim = (h', w) flat (64*128 = 8192).  Loads from
    DRAM are 128 contiguous 32KB descriptors with SWDGE f32->fp16 cast.
  - All 9 (dh, dw) shifts are free-dim offsets delta = 128*dh + dw into a
    zero-padded copy of feature2.  A second copy shifted by one element
    keeps odd-delta operands 4-byte aligned so DVE fp16 muls run at 2x.
  - Channel reduction via TensorE: matmul with a sliding one-hot
    column-pair mask [128, 128] (ones over partitions 0-63 in col 2k,
    over 64-127 in col 2k+1) reduces chunk k of a product over c and
    drops row sums into PSUM rows (2k, 2k+1); 64 accumulating matmuls
    per map fill a [128, 128] fp32 score tile laid out as
    [p = 2h'+hh, w].  11 maps: 9 correlations + |f1|^2 + |f2|^2.
  - L2 normalization folded into score scaling:
      score_d = 10 * A_d * rsqrt(|f1|^2) * rsqrt(|f2|^2 shifted)
    Scores bounded by 10 so softmax needs no max subtraction.
"""

import numpy as np

B, C, H, W = 8, 64, 128, 128
N_CORES = 8
SOFTMAX_SCALE = 10.0

HH = 2
HP = H // HH            # 64 h-rows per half
FREE = HP * W           # 8192 free elems per partition
M0 = 160                # main-data column offset in padded f2 tiles
F2W = M0 + FREE + 160   # 8512

_CACHE = {}


def _build_program():
    import concourse.bass as bass
    import concourse.bacc as bacc
    import concourse.mybir as mybir
    from concourse.tile import TileContext
    from concourse.bass_utils import axon_active

    f32 = mybir.dt.float32
    f16 = mybir.dt.float16
    AF = mybir.ActivationFunctionType
    red = dict(axis=mybir.AxisListType.X, op=mybir.AluOpType.add)

    nc = bacc.Bacc(
        "TRN2",
        target_bir_lowering=False,
        debug=not axon_active(),
        num_devices=N_CORES,
    )

    f1d = nc.declare_dram_parameter("feature1", [C, H, W], f32, isOutput=False)
    f2d = nc.declare_dram_parameter("feature2", [C, H, W], f32, isOutput=False)
    outd = nc.declare_dram_parameter("flow", [2, H, W], f32, isOutput=True)

    # [64, 8192] per-half views: partition c, free = h'*W + w
    f1h = [
        f1d[:, hh * HP : (hh + 1) * HP, :].rearrange("c hp w -> c (hp w)")
        for hh in range(HH)
    ]
    f2h = [
        f2d[:, hh * HP : (hh + 1) * HP, :].rearrange("c hp w -> c (hp w)")
        for hh in range(HH)
    ]
    # output view: partition = h (score layout), free = (o, w)
    outv = outd.rearrange("o h w -> h o w")

    with TileContext(nc) as tc:
        with tc.tile_pool(name="main", bufs=1) as pool, \
             tc.tile_pool(name="prod", bufs=6) as prodp, \
             tc.tile_pool(name="psum", bufs=1, space="PSUM") as psp:

            # ---- input tiles (fp16, cast during SWDGE DMA) ----
            xf1 = pool.tile([128, FREE], f16)
            xf2 = pool.tile([128, F2W], f16)    # even-parity padded f2
            xf2o = pool.tile([128, F2W], f16)   # same, shifted 1 elem left

            nc.gpsimd.dma_start(out=xf1[0:64, :], in_=f1h[0])
            nc.gpsimd.dma_start(out=xf1[64:128, :], in_=f1h[1])

            # zero the pad regions (h'=-1 / h'=64 halos + spare)
            nc.vector.memset(xf2[:, 0:M0], 0.0)
            nc.vector.memset(xf2[:, M0 + FREE : F2W], 0.0)
            nc.vector.memset(xf2o[:, 0 : M0 - 1], 0.0)
            nc.vector.memset(xf2o[:, M0 - 1 + FREE : F2W], 0.0)

            # main f2 data
            nc.gpsimd.dma_start(out=xf2[0:64, M0 : M0 + FREE], in_=f2h[0])
            nc.gpsimd.dma_start(out=xf2[64:128, M0 : M0 + FREE], in_=f2h[1])
            nc.sync.dma_start(
                out=xf2o[0:64, M0 - 1 : M0 - 1 + FREE],
                in_=xf2[0:64, M0 : M0 + FREE],
            )
            nc.sync.dma_start(
                out=xf2o[64:128, M0 - 1 : M0 - 1 + FREE],
                in_=xf2[64:128, M0 : M0 + FREE],
            )
            # halo rows: hh=0 needs h=64 above its top edge is h=-1 (zero),
            # below its last row h'=63 is h=64; hh=1 has h=63 above, zero below.
            nc.gpsimd.dma_start(
                out=xf2[64:128, M0 - 128 : M0], in_=f2d[:, HP - 1, :]
            )
            nc.gpsimd.dma_start(
                out=xf2[0:64, M0 + FREE : M0 + FREE + 128], in_=f2d[:, HP, :]
            )
            nc.sync.dma_start(
                out=xf2o[64:128, M0 - 129 : M0 - 1],
                in_=xf2[64:128, M0 - 128 : M0],
            )
            nc.sync.dma_start(
                out=xf2o[0:64, M0 - 1 + FREE : M0 - 1 + FREE + 128],
                in_=xf2[0:64, M0 + FREE : M0 + FREE + 128],
            )

            # ---- sliding one-hot mask for the channel-reduce matmuls ----
            # S_k = smask[:, 63-k : 191-k]:
            #   S_k[p, k]    = 1 for p in [0,64)   -> PSUM row k    = h = k
            #   S_k[p, 64+k] = 1 for p in [64,128) -> row 64+k      = h = 64+k
            smask = pool.tile([128, 191], f16)
            nc.vector.memset(smask[:, :], 0.0)
            nc.vector.memset(smask[0:64, 63:64], 1.0)
            nc.vector.memset(smask[64:128, 127:128], 1.0)

            # ---- PSUM score tiles: one full bank per concurrently
            # accumulating map (start=True clears has_written bank-wide,
            # so interleaved accumulation groups must not share a bank).
            ptiles = [
                psp.tile([128, 128], f32, tag=f"pb{i}", name=f"pb{i}")
                for i in range(8)
            ]
            # s_all score columns: d*128 for d=0..8, |f1|^2 @1152, |f2|^2 @1280
            s_all = pool.tile([128, 1408], f32)
            SCOL = {d: d * 128 for d in range(9)}
            SCOL[9], SCOL[10] = 1152, 1280

            def shift_view(dh, dw):
                delta = 128 * dh + dw
                if dw == 0:
                    return xf2[:, M0 + delta : M0 + delta + FREE]
                return xf2o[:, M0 + delta - 1 : M0 + delta - 1 + FREE]

            groups = [[9, 10, 4, 1], [7, 3, 5, 0], [2, 6, 8]]
            gbanks = [ptiles[0:4], ptiles[4:8], ptiles[0:3]]
            for gi, g in enumerate(groups):
                banks = gbanks[gi]
                prs = []
                for m in g:
                    pr = prodp.tile([128, FREE], f16, tag="prod")
                    if m == 9:
                        nc.scalar.activation(pr[:, :], xf1[:, :], AF.Square)
                    elif m == 10:
                        nc.scalar.activation(
                            pr[:, :], xf2[:, M0 : M0 + FREE], AF.Square
                        )
                    else:
                        dh, dw = m // 3 - 1, m % 3 - 1
                        nc.vector.tensor_mul(
                            pr[:, :], xf1[:, :], shift_view(dh, dw)
                        )
                    prs.append(pr)
                for k in range(HP):
                    lhs = smask[:, 63 - k : 191 - k]
                    for pr, bank in zip(prs, banks):
                        nc.tensor.matmul(
                            bank[:, :],
                            lhs,
                            pr[:, 128 * k : 128 * (k + 1)],
                            start=(k == 0),
                            stop=(k == HP - 1),
                        )
                # drain this group's scores to SBUF (frees the banks and
                # lets the norm/rsqrt chain start after group 0)
                for m, bank in zip(g, banks):
                    nc.scalar.copy(
                        s_all[:, SCOL[m] : SCOL[m] + 128], bank[:, :]
                    )
            sc = s_all[:, 0:1152]

            # ---- rsqrt of norms ----
            r1 = pool.tile([128, 128], f32)
            r2p = pool.tile([128, 130], f32)
            nc.scalar.sqrt(r1[:, :], s_all[:, 1152:1280])
            nc.vector.reciprocal(r1[:, :], r1[:, :])
            nc.vector.memset(r2p[:, 0:1], 1.0)
            nc.vector.memset(r2p[:, 129:130], 1.0)
            nc.scalar.sqrt(r2p[:, 1:129], s_all[:, 1280:1408])
            nc.vector.reciprocal(r2p[:, 1:129], r2p[:, 1:129])

            # dh-shifted copies of r2 (partition = h, shift by one).
            # Edge rows clamp (their A is exactly 0).
            r2hp = pool.tile([128, 130], f32)
            r2hm = pool.tile([128, 130], f32)
            nc.sync.dma_start(out=r2hp[0:127, :], in_=r2p[1:128, :])
            nc.sync.dma_start(out=r2hp[127:128, :], in_=r2p[127:128, :])
            nc.sync.dma_start(out=r2hm[1:128, :], in_=r2p[0:127, :])
            nc.sync.dma_start(out=r2hm[0:1, :], in_=r2p[0:1, :])

            # scores -> normalized scores (in place):
            # shat_d[h, w] = A_d[h, w] * r1[h, w] * r2[h+dh, w+dw]
            for d in range(9):
                dh, dw = d // 3 - 1, d % 3 - 1
                r2x = (r2hm, r2p, r2hp)[dh + 1]
                sd = sc[:, d * 128 : (d + 1) * 128]
                nc.vector.tensor_mul(sd, sd, r2x[:, 1 + dw : 129 + dw])
                nc.vector.tensor_mul(sd, sd, r1[:, :])
            # w-edge shifts wrap across rows: those scores are exactly 0
            # in the reference (zero padding), so overwrite them.
            for d in (0, 3, 6):
                nc.vector.memset(sc[:, d * 128 : d * 128 + 1], 0.0)
            for d in (2, 5, 8):
                nc.vector.memset(sc[:, d * 128 + 127 : d * 128 + 128], 0.0)

            # ---- softmax-weighted displacement sums ----
            expo = pool.tile([128, 1152], f32)
            nc.scalar.activation(
                expo[:, :], sc, AF.Exp, scale=SOFTMAX_SCALE
            )

            esum = pool.tile([128, 128], f32)
            fwp = pool.tile([128, 128], f32)
            fwm = pool.tile([128, 128], f32)
            fhp = pool.tile([128, 128], f32)
            fhm = pool.tile([128, 128], f32)
            ex4 = expo.rearrange("p (a b w) -> p a b w", a=3, b=3)
            nc.vector.tensor_reduce(
                esum[:, :], expo.rearrange("p (d w) -> p w d", d=9), **red
            )
            nc.vector.tensor_reduce(
                fwp[:, :], ex4[:, :, 2, :].rearrange("p a w -> p w a"), **red
            )
            nc.vector.tensor_reduce(
                fwm[:, :], ex4[:, :, 0, :].rearrange("p a w -> p w a"), **red
            )
            nc.vector.tensor_reduce(
                fhp[:, :], ex4[:, 2, :, :].rearrange("p b w -> p w b"), **red
            )
            nc.vector.tensor_reduce(
                fhm[:, :], ex4[:, 0, :, :].rearrange("p b w -> p w b"), **red
            )

            flows = pool.tile([128, 2, W], f32)
            nc.vector.reciprocal(esum[:, :], esum[:, :])
            nc.vector.tensor_sub(fwp[:, :], fwp[:, :], fwm[:, :])
            nc.vector.tensor_sub(fhp[:, :], fhp[:, :], fhm[:, :])
            nc.vector.tensor_mul(flows[:, 0, :], fwp[:, :], esum[:, :])
            nc.vector.tensor_mul(flows[:, 1, :], fhp[:, :], esum[:, :])

            nc.sync.dma_start(out=outv, in_=flows[:, :, :])

    nc.compile()
    return nc


LAST_RESULT = None


def kernel(feature1: np.ndarray, feature2: np.ndarray) -> np.ndarray:
    global LAST_RESULT
    from concourse import bass_utils

    if "nc" not in _CACHE:
        _CACHE["nc"] = _build_program()
    nc = _CACHE["nc"]

    f1 = np.ascontiguousarray(np.asarray(feature1, dtype=np.float32))
    f2 = np.ascontiguousarray(np.asarray(feature2, dtype=np.float32))
    in_maps = [
        {"feature1": f1[b], "feature2": f2[b]} for b in range(N_CORES)
    ]
    res = bass_utils.run_bass_kernel_spmd(nc, in_maps, list(range(N_CORES)))
    LAST_RESULT = res
    out = np.stack([res.results[b]["flow"] for b in range(N_CORES)], axis=0)
    return out.astype(np.float32)
